# revision 10
# baseline (speedup 1.0000x reference)
"""Trainium2 Bass kernel for nn_AttentionDot (double batch-axis softmax attention).

Computation (B=4, N=M=4096, D=128, fp32):
    scores[b,n,m] = sum_d k[b,n,d] * q[b,m,d]
    w = softmax(softmax(scores, axis=0), axis=0)      # over batch axis (size 4)
    out[b,n,d]  = sum_m w[b,n,m] * v[b,m,d]

Sharding: N (rows of k / rows of scores) split across 8 NeuronCores; q, v
replicated. Each core computes its [B, 512, M] score slab, the axis-0
softmax (local — all 4 batch entries present), and its [B, 512, D] output
slab independently. No collectives.

Per-core layout: scores are built transposed, s_T[b] tiles [128(m), 512(n)],
so the second matmul (contract over m) takes w_T directly as the moving
operand and v in natural [m, d] layout as the stationary operand, producing
out_T [d, n] accumulated in PSUM over the 32 m-chunks.

Softmax over b is elementwise across (m, n): exp on ScalarE; the sums over
the 4 batch tiles run on the PE as accumulating identity matmuls (frees the
VectorE, which only does the two normalization multiplies + reciprocals).
No max-subtraction: |scores| <~ 70 so exp stays in fp32 range, and the
second softmax's inputs are in (0,1).

Execution: each core runs the same single-device NEFF via its own PJRT
dispatch (async, all 8 in flight) — the multi-device shard_map executable
path wedges the axon terminal, so it is deliberately avoided.
"""

import numpy as np

import concourse.bass as bass
import concourse.tile as tile
from concourse import bacc, mybir, masks

B, N, M, D = 4, 4096, 4096, 128
NCORES = 8
NSH = N // NCORES            # 512 k-rows per core
NSUB = NSH // 128            # 4 partition-tiles of n
MCH = M // 128               # 32 m-chunks

F32 = mybir.dt.float32
F32R = mybir.dt.float32r
BF16 = mybir.dt.bfloat16
EXP = mybir.ActivationFunctionType.Exp

# float32r runs the PE at 1 cycle/row (vs 4 for plain fp32) when the moving
# free dim is >= 256, at slightly reduced mantissa precision.
MM_DT = F32R
# dtype of the softmax elementwise chain (exp outputs / muls). BF16 gives the
# VectorE its 2x packed mode at ~0.5% relative error; F32 is exact-ish.
SOFT_DT = BF16


def _mm(ap):
    """View an fp32 AP with the matmul compute dtype."""
    if MM_DT is F32:
        return ap
    return ap.bitcast(MM_DT)


def _soft_mm(ap):
    """Softmax-chain tile as a matmul input."""
    if SOFT_DT is BF16:
        return ap
    return _mm(ap)


def build_nc():
    nc = bacc.Bacc(
        "TRN2",
        target_bir_lowering=False,
        debug=False,
        enable_asserts=False,
        num_devices=NCORES,
    )
    kk = nc.dram_tensor("k", [B, NSH, D], F32, kind="ExternalInput").ap()
    qq = nc.dram_tensor("q", [B, M, D], F32, kind="ExternalInput").ap()
    vv = nc.dram_tensor("v", [B, M, D], F32, kind="ExternalInput").ap()
    out = nc.dram_tensor("out", [B, NSH, D], F32, kind="ExternalOutput").ap()

    from contextlib import ExitStack

    with tile.TileContext(nc) as tc, ExitStack() as ctx:
        const_pool = ctx.enter_context(tc.tile_pool(name="const", bufs=1))
        ident = const_pool.tile([128, 128], F32)
        masks.make_identity(nc, ident[:])
        if SOFT_DT is BF16:
            identb = const_pool.tile([128, 128], BF16)
            nc.vector.tensor_copy(identb[:], ident[:])
            sum_lhsT = identb[:]
        else:
            sum_lhsT = _mm(ident[:])

        big = ctx.enter_context(tc.tile_pool(name="big", bufs=1))
        qT = big.tile([128, B, M], F32, tag="qT")            # [d, b, m] 64KB/p
        vS = big.tile([128, B, MCH, 128], SOFT_DT, tag="v")  # [m_sub, b, c, d]
        kT = big.tile([128, B, NSH], F32, tag="kT")          # [d, b, n] 8KB/p

        psA = ctx.enter_context(tc.tile_pool(name="psA", bufs=4, space="PSUM"))
        psO = ctx.enter_context(tc.tile_pool(name="psO", bufs=1, space="PSUM"))

        # ---- v: natural layout loads --------------------------------------
        for b in range(B):
            if SOFT_DT is F32:
                nc.sync.dma_start(
                    vS[:, b], vv[b].rearrange("(c p) d -> p c d", p=128)
                )
            else:
                nc.gpsimd.dma_start(
                    vS[:, b], vv[b].rearrange("(c p) d -> p c d", p=128)
                )

        # ---- k, q: load natural, transpose via PE -------------------------
        with tc.tile_pool(name="stage", bufs=2) as stage:
            for b in range(B):
                knat = stage.tile([128, NSUB, 128], F32, tag="knat")
                nc.sync.dma_start(
                    knat[:], kk[b].rearrange("(c p) d -> p c d", p=128)
                )
                ps = psA.tile([128, 512], F32, tag="s", name=f"kt{b}")
                for j in range(NSUB):
                    nc.tensor.transpose(
                        ps[:, j * 128 : (j + 1) * 128], knat[:, j], ident[:]
                    )
                nc.scalar.copy(_mm(kT[:, b]), ps[:])
            for b in range(B):
                for g in range(MCH // 4):       # 8 groups of 4 m-chunks
                    qnat = stage.tile([128, 4, 128], F32, tag="qnat")
                    nc.sync.dma_start(
                        qnat[:],
                        qq[b, g * 512 : (g + 1) * 512].rearrange(
                            "(c p) d -> p c d", p=128
                        ),
                    )
                    ps = psA.tile([128, 512], F32, tag="s", name=f"qt{b}_{g}")
                    for j in range(4):
                        nc.tensor.transpose(
                            ps[:, j * 128 : (j + 1) * 128],
                            qnat[:, j],
                            ident[:],
                        )
                    if g % 2 == 0:
                        nc.scalar.copy(_mm(qT[:, b, g * 512 : (g + 1) * 512]), ps[:])
                    else:
                        nc.vector.tensor_copy(_mm(qT[:, b, g * 512 : (g + 1) * 512]), ps[:])

        # ---- main loop over m-chunks --------------------------------------
        # out_T accumulators [d, n] per b, accumulated across all chunks.
        outps = [
            psO.tile([128, 512], F32, tag=f"o{b}", name=f"outps{b}") for b in range(B)
        ]

        with tc.tile_pool(name="soft", bufs=3) as soft, tc.tile_pool(
            name="stat", bufs=2
        ) as stat:
            for c in range(MCH):
                # scores_T[b] = qT_chunk^T(d,m) x kT[b](d,n) -> [m, n] in PSUM
                sps = []
                for b in range(B):
                    sp = psA.tile([128, 512], F32, tag="s", name=f"s{c}_{b}")
                    nc.tensor.matmul(
                        sp[:],
                        _mm(qT[:, b, c * 128 : (c + 1) * 128]),
                        _mm(kT[:, b]),
                        start=True,
                        stop=True,
                    )
                    sps.append(sp)

                # softmax over b (elementwise in (m, n)), twice.
                # e: [128, B, 512] holds the 4 batch slabs contiguously.
                e = soft.tile([128, B, 512], SOFT_DT, tag="e", name=f"e{c}")
                for b in range(B):
                    nc.scalar.activation(e[:, b], sps[b][:], EXP)
                # S = sum_b e_b via accumulating identity matmuls on the PE
                ssum = psA.tile([128, 512], F32, tag="s", name=f"ss{c}")
                for b in range(B):
                    nc.tensor.matmul(
                        ssum[:], sum_lhsT, _soft_mm(e[:, b]),
                        start=(b == 0), stop=(b == 3),
                    )
                rcp = stat.tile([128, 512], F32, tag="t0", name=f"r{c}")
                nc.vector.reciprocal_approx_fast(rcp[:], ssum[:])
                if SOFT_DT is BF16:
                    rcpc = stat.tile([128, 512], BF16, tag="t0b", name=f"rb{c}")
                    nc.vector.tensor_copy(rcpc[:], rcp[:])
                else:
                    rcpc = rcp
                # y = e * S^-1 (broadcast over b), then g = exp(y), in place
                rb = rcpc[:].unsqueeze(1).broadcast_to([128, B, 512])
                nc.vector.tensor_mul(e[:], e[:], rb)
                nc.scalar.activation(e[:], e[:], EXP)
                tsum = psA.tile([128, 512], F32, tag="s", name=f"ts{c}")
                for b in range(B):
                    nc.tensor.matmul(
                        tsum[:], sum_lhsT, _soft_mm(e[:, b]),
                        start=(b == 0), stop=(b == 3),
                    )
                tcp = stat.tile([128, 512], F32, tag="t1", name=f"t{c}")
                nc.vector.reciprocal_approx_fast(tcp[:], tsum[:])
                if SOFT_DT is BF16:
                    tcpc = stat.tile([128, 512], BF16, tag="t1b", name=f"tb{c}")
                    nc.vector.tensor_copy(tcpc[:], tcp[:])
                else:
                    tcpc = tcp
                tb = tcpc[:].unsqueeze(1).broadcast_to([128, B, 512])
                nc.vector.tensor_mul(e[:], e[:], tb)   # w

                # out_T[b] += v_chunk[b]^T(m,d) x w_b(m,n) -> [d, n]
                for b in range(B):
                    nc.tensor.matmul(
                        outps[b][:],
                        vS[:, b, c] if SOFT_DT is BF16 else _mm(vS[:, b, c]),
                        _soft_mm(e[:, b]),
                        start=(c == 0),
                        stop=(c == MCH - 1),
                    )

        # ---- epilogue: transpose out_T [d, n] -> [n, d], store ------------
        with tc.tile_pool(name="epi", bufs=4) as epi:
            for b in range(B):
                osb = epi.tile([128, 512], F32, tag="osb", name=f"osb{b}")
                nc.scalar.copy(osb[:], outps[b][:])
                ps = psA.tile([128, 512], F32, tag="s", name=f"ot{b}")
                for j in range(NSUB):
                    nc.tensor.transpose(
                        ps[:, j * 128 : (j + 1) * 128],
                        osb[:, j * 128 : (j + 1) * 128],
                        ident[:],
                    )
                onat = epi.tile([128, NSUB, 128], F32, tag="onat", name=f"onat{b}")
                nc.vector.tensor_copy(
                    onat[:], ps[:].rearrange("p (j d) -> p j d", j=NSUB)
                )
                nc.sync.dma_start(
                    out[b].rearrange("(j p) d -> p j d", p=128), onat[:]
                )

    nc.compile()
    return nc


# ---------------------------------------------------------------------------
# host-side execution

_NC_CACHE = None
LAST_RESULTS = None
LAST_EXEC_NS = None
LAST_PATH = None


def _with_timeout(fn, secs):
    """Run fn in a daemon thread with a deadline; raises TimeoutError.
    A hung remote fetch cannot be cancelled — the thread is leaked."""
    import threading

    box = {}

    def run():
        try:
            box["val"] = fn()
        except BaseException as e:  # noqa: BLE001
            box["err"] = e

    th = threading.Thread(target=run, daemon=True)
    th.start()
    th.join(secs)
    if "val" in box:
        return box["val"]
    if "err" in box:
        raise box["err"]
    raise TimeoutError(f"timed out after {secs}s")


def _run_spmd_native(nc, in_maps):
    """Native hardware path (real /dev/neuron*): the stock 8-core runner."""
    from concourse.bass_utils import run_bass_kernel_spmd

    res = run_bass_kernel_spmd(nc, in_maps, core_ids=list(range(NCORES)))
    global LAST_EXEC_NS
    if res.exec_time_ns is not None:
        LAST_EXEC_NS = res.exec_time_ns
    return res.results


def _run_per_device_axon(nc, in_maps):
    """Axon path: run the (collective-free) NEFF on each core as an
    independent single-device PJRT execution via the stock 1-core runner.
    The 8-device shard_map executable is avoided (it can wedge the axon
    terminal). Device 0 doubles as the compile probe: if it doesn't come
    back within its budget the whole path is abandoned."""
    import jax
    from concourse import bass2jax

    devs = jax.devices()
    results = []
    for c in range(NCORES):
        def call(c=c):
            with jax.default_device(devs[c]):
                return bass2jax.run_bass_via_pjrt(nc, [in_maps[c]], n_cores=1)

        # first call pays the NEFF compile; later calls reuse the cache
        results.append(_with_timeout(call, 1200 if c == 0 else 240)[0])
    return results


def _run_coresim(nc, in_maps):
    """Pure-simulation fallback: numerically correct, no hardware."""
    from concourse.bass_interp import CoreSim

    results = []
    for c in range(NCORES):
        sim = CoreSim(nc, trace=False, require_finite=False, require_nnan=False)
        for name, arr in in_maps[c].items():
            sim.tensor(name)[:] = arr
        sim.simulate(check_with_hw=False)
        results.append({"out": np.array(sim.tensor("out"))})
    return results


def kernel(k, q, v, _trace=False):
    global _NC_CACHE, LAST_RESULTS, LAST_PATH
    k = np.ascontiguousarray(np.asarray(k, dtype=np.float32))
    q = np.ascontiguousarray(np.asarray(q, dtype=np.float32))
    v = np.ascontiguousarray(np.asarray(v, dtype=np.float32))
    assert k.shape == (B, N, D) and q.shape == (B, M, D) and v.shape == (B, M, D)

    if _NC_CACHE is None:
        _NC_CACHE = build_nc()
    nc = _NC_CACHE

    in_maps = [
        {
            "k": np.ascontiguousarray(k[:, i * NSH : (i + 1) * NSH, :]),
            "q": q,
            "v": v,
        }
        for i in range(NCORES)
    ]

    from concourse._compat import axon_active

    attempts = []
    if axon_active():
        attempts.append(("axon-per-device", lambda: _run_per_device_axon(nc, in_maps), 2400))
    else:
        attempts.append(("native-spmd", lambda: _run_spmd_native(nc, in_maps), 2400))

    results = None
    for name, fn, budget in attempts:
        try:
            results = _with_timeout(fn, budget)
            LAST_PATH = name
            break
        except BaseException as e:  # noqa: BLE001
            import sys

            print(f"kernel: {name} failed ({e!r}); falling back", file=sys.stderr)
    if results is None:
        results = _run_coresim(nc, in_maps)
        LAST_PATH = "coresim"

    LAST_RESULTS = results
    return np.concatenate([r["out"] for r in results], axis=1)
